# revision 1
# baseline (speedup 1.0000x reference)
"""Trainium2 Bass kernel for modality-routed (CogVLM-style) attention.

Contract: kernel(**inputs) takes FULL unsharded numpy inputs (as produced by
the reference's setup_inputs) and returns the FULL [2048, 4096] fp32 output.

Sharding: tensor-parallel over heads. Core r owns heads 4r..4r+3:
  - qkv weights column-sharded: q/k slices used as matmul lhsT (stationary),
    producing qT/kT directly in [D, S] orientation (no transposes on device);
    v computed in natural [S, d] orientation.
  - dense weights row-sharded [512, 4096]; each core emits a partial
    [2048, 4096] output, summed on the host (the unshard step).
Routing (vision tokens = rows 0..NV-1, language = NV..S-1) is handled by
splitting matmuls at the NV boundary; the vision qkv bias is fused into the
PSUM->SBUF evacuations.

All PE matmuls run in float32r (fp32 data, ~1.4e-4 rel err, full rate on TRN2
for N>=256). Softmax skips the max-subtraction (scores are O(10) here; exact
same math), computes probsT = exp(scoresT) tile-by-tile with causal block
skipping + a triangular mask on diagonal blocks, row sums via a ones-vector
matmul, and folds 1/sum into the attnT evacuation via a DMA-broadcast row.
"""

import sys

import numpy as np

if "/opt/trn_rl_repo" not in sys.path:
    sys.path.insert(0, "/opt/trn_rl_repo")

import concourse.bass as bass  # noqa: E402,F401
import concourse.tile as tile  # noqa: E402
from concourse import bacc, mybir  # noqa: E402
from concourse.bass_utils import run_bass_kernel_spmd  # noqa: E402

S = 2048
HID = 4096
H = 32
D = 128
NCORES = 8
HPC = H // NCORES          # heads per core = 4
QKC = 2 * HPC * D          # q+k outdim rows per core = 1024
VC = HPC * D               # v outdim per core = 512
NV = 576                   # vision tokens occupy rows [0, NV)
NKT = HID // 128           # 32 K-tiles

F32 = mybir.dt.float32
F32R = mybir.dt.float32r

_CACHE = {}


def _chunks():
    # (c0, c1, expert, side64): first vision chunk carries tokens 512..576
    # as a side segment so they share the same weight-tile stream; language
    # chunks are 128-aligned.
    return [(0, 512, "V", True), (NV, 1024, "L", False),
            (1024, 1536, "L", False), (1536, 2048, "L", False)]


def _token_tiles(t0, t1):
    out = []
    c = t0
    while c < t1:
        n = min(t1, (c // 128 + 1) * 128)
        out.append((c, n))
        c = n
    return out


def _build():
    nc = bacc.Bacc("TRN2", target_bir_lowering=False, debug=False,
                   num_devices=NCORES)
    dti = nc.dram_tensor
    hsT = dti("hsT", [HID, S], F32R, kind="ExternalInput").ap()
    wqk_v = dti("wqk_v", [HID, QKC], F32R, kind="ExternalInput").ap()
    wqk_l = dti("wqk_l", [HID, QKC], F32R, kind="ExternalInput").ap()
    wv_v = dti("wv_v", [HID, VC], F32R, kind="ExternalInput").ap()
    wv_l = dti("wv_l", [HID, VC], F32R, kind="ExternalInput").ap()
    wd_v = dti("wd_v", [VC, HID], F32R, kind="ExternalInput").ap()
    wd_l = dti("wd_l", [VC, HID], F32R, kind="ExternalInput").ap()
    bqk = dti("bqk", [128, 8], F32, kind="ExternalInput").ap()
    bv = dti("bv", [1, VC], F32, kind="ExternalInput").ap()
    cosq = dti("cosq", [D, S], F32, kind="ExternalInput").ap()
    sinq = dti("sinq", [D, S], F32, kind="ExternalInput").ap()
    cosk = dti("cosk", [D, S], F32, kind="ExternalInput").ap()
    sink = dti("sink", [D, S], F32, kind="ExternalInput").ap()
    rmT = dti("rmT", [D, D], F32R, kind="ExternalInput").ap()
    ones = dti("ones", [128, 1], F32R, kind="ExternalInput").ap()
    masks = dti("masks", [128, 4 * 512], F32R, kind="ExternalInput").ap()
    qkr_d = dti("qkr", [QKC, S], F32R).ap()          # roped qT/kT scratch
    v_d = dti("vsc", [S, VC], F32R).ap()             # v scratch
    rcp_d = dti("rcp", [HPC, S], F32).ap()           # softmax 1/sum rows
    out_d = dti("out", [S, HID], F32, kind="ExternalOutput").ap()

    CH = _chunks()
    with tile.TileContext(nc) as tc:
        with tc.tile_pool(name="glob", bufs=1) as glob:
            ones_t = glob.tile([128, 1], F32R)
            nc.sync.dma_start(out=ones_t[:], in_=ones[:])
            mask_t = glob.tile([128, 4 * 512], F32R)
            nc.sync.dma_start(out=mask_t[:], in_=masks[:])

            # ---------------- QKV phase ----------------
            with tc.tile_pool(name="consts", bufs=1) as consts, \
                 tc.tile_pool(name="hs", bufs=1) as hs_pool, \
                 tc.tile_pool(name="wq", bufs=2) as wq_pool, \
                 tc.tile_pool(name="wvp", bufs=1) as wv_pool, \
                 tc.tile_pool(name="ev", bufs=2) as ev_pool, \
                 tc.tile_pool(name="ps", bufs=2, space="PSUM") as ps, \
                 tc.tile_pool(name="ps1", bufs=2, space="PSUM") as ps1:
                bqk_t = consts.tile([128, 8], F32)
                nc.sync.dma_start(out=bqk_t[:], in_=bqk[:])
                bv_t = consts.tile([128, VC], F32)
                nc.sync.dma_start(out=bv_t[:], in_=bv[:].to_broadcast((128, VC)))
                rm_t = consts.tile([D, D], F32R)
                nc.sync.dma_start(out=rm_t[:], in_=rmT[:])

                for (c0, c1, e, side) in CH:
                    w = c1 - c0
                    cw = (NV - c0) if side else w       # cos/sin span
                    wqk = wqk_v if e == "V" else wqk_l
                    wv = wv_v if e == "V" else wv_l
                    hst = hs_pool.tile([128, NKT, 512], F32R, tag="hst")
                    for kt in range(NKT):
                        nc.sync.dma_start(
                            out=hst[:, kt, :w],
                            in_=hsT[128 * kt:128 * (kt + 1), c0:c1])
                    h64 = None
                    if side:
                        h64 = hs_pool.tile([128, NKT, 64], F32R, tag="h64")
                        for kt in range(NKT):
                            nc.sync.dma_start(
                                out=h64[:, kt, :],
                                in_=hsT[128 * kt:128 * (kt + 1), 512:NV])
                    cs = []
                    for tag, src in (("cqc", cosq), ("sqc", sinq),
                                     ("ckc", cosk), ("skc", sink)):
                        t = consts.tile([128, 576], F32, tag=tag)
                        nc.sync.dma_start(out=t[:, :cw], in_=src[:, c0:c0 + cw])
                        cs.append(t)
                    segs = [(c0, c1)] + ([(512, NV)] if side else [])
                    # q/k rows: m<HPC -> q head m, m>=HPC -> k head m-HPC
                    for m in range(2 * HPC):
                        wt = wq_pool.tile([128, NKT, 128], F32R, tag="wt")
                        for kt in range(NKT):
                            nc.sync.dma_start(
                                out=wt[:, kt, :],
                                in_=wqk[128 * kt:128 * (kt + 1),
                                        128 * m:128 * (m + 1)])
                        cos_c = cs[0] if m < HPC else cs[2]
                        sin_c = cs[1] if m < HPC else cs[3]
                        for (a0, a1) in segs:
                            w2 = a1 - a0
                            is64 = side and a0 >= 512
                            src = h64 if is64 else hst
                            o0 = a0 - c0
                            pt = ps.tile([128, 512], F32, tag="qk_ps")
                            for kt in range(NKT):
                                nc.tensor.matmul(pt[:, :w2], wt[:, kt, :],
                                                 src[:, kt, :w2],
                                                 start=(kt == 0),
                                                 stop=(kt == NKT - 1))
                            qk_sb = ev_pool.tile([128, 512], F32R, tag="qk_sb")
                            if e == "V":
                                nc.scalar.activation(
                                    out=qk_sb[:, :w2], in_=pt[:, :w2],
                                    func=mybir.ActivationFunctionType.Identity,
                                    bias=bqk_t[:, m:m + 1], scale=1.0)
                            else:
                                nc.scalar.activation(
                                    out=qk_sb[:, :w2], in_=pt[:, :w2],
                                    func=mybir.ActivationFunctionType.Copy,
                                    scale=1.0)
                            rot = ps1.tile([128, 512], F32, tag="rot_ps")
                            nc.tensor.matmul(rot[:, :w2], rm_t[:],
                                             qk_sb[:, :w2],
                                             start=True, stop=True)
                            tb = ev_pool.tile([128, 512], F32, tag="tb")
                            nc.vector.tensor_mul(tb[:, :w2], rot[:, :w2],
                                                 sin_c[:, o0:o0 + w2])
                            nc.vector.tensor_mul(qk_sb[:, :w2], qk_sb[:, :w2],
                                                 cos_c[:, o0:o0 + w2])
                            rr = ev_pool.tile([128, 512], F32R, tag="rr")
                            nc.vector.tensor_add(rr[:, :w2], qk_sb[:, :w2],
                                                 tb[:, :w2])
                            nc.sync.dma_start(
                                out=qkr_d[128 * m:128 * (m + 1), a0:a1],
                                in_=rr[:, :w2])
                    # v in natural [token, d] orientation
                    wvt = wv_pool.tile([128, NKT, VC], F32R, tag="wvt")
                    for kt in range(NKT):
                        nc.sync.dma_start(
                            out=wvt[:, kt, :],
                            in_=wv[128 * kt:128 * (kt + 1), :])
                    tts = _token_tiles(c0, c1) + ([(512, NV)] if side else [])
                    for (t0, t1) in tts:
                        mw = t1 - t0
                        src = h64 if t0 >= 512 and side else hst
                        off = 512 if (t0 >= 512 and side) else c0
                        pv = ps.tile([128, VC], F32, tag="v_ps")
                        for kt in range(NKT):
                            nc.tensor.matmul(
                                pv[:mw, :],
                                src[:, kt, t0 - off:t1 - off],
                                wvt[:, kt, :],
                                start=(kt == 0), stop=(kt == NKT - 1))
                        v_sb = ev_pool.tile([128, VC], F32R, tag="v_sb")
                        if e == "V":
                            nc.vector.tensor_add(v_sb[:mw, :], pv[:mw, :],
                                                 bv_t[:mw, :])
                        else:
                            nc.vector.tensor_copy(v_sb[:mw, :], pv[:mw, :])
                        nc.sync.dma_start(out=v_d[t0:t1, :], in_=v_sb[:mw, :])

            # ---------------- attention phase ----------------
            with tc.tile_pool(name="an", bufs=1) as an_pool:
                attn = []
                for h in range(HPC):
                    a_t = an_pool.tile([128, S], F32R, tag=f"attnT{h}")
                    attn.append(a_t)
                with tc.tile_pool(name="att", bufs=1) as att, \
                     tc.tile_pool(name="pr", bufs=3) as pr_pool, \
                     tc.tile_pool(name="sm", bufs=2) as sm_pool, \
                     tc.tile_pool(name="ps2", bufs=3, space="PSUM") as ps2, \
                     tc.tile_pool(name="ps3", bufs=2, space="PSUM") as ps3, \
                     tc.tile_pool(name="ps4", bufs=2, space="PSUM") as ps4:
                    for h in range(HPC):
                        qt = att.tile([128, S], F32R, tag="qt")
                        kt_ = att.tile([128, S], F32R, tag="kt")
                        vt = att.tile([128, 16, 128], F32R, tag="vt")
                        nc.sync.dma_start(
                            out=qt[:], in_=qkr_d[128 * h:128 * (h + 1), :])
                        nc.sync.dma_start(
                            out=kt_[:],
                            in_=qkr_d[VC + 128 * h:VC + 128 * (h + 1), :])
                        for tt in range(16):
                            nc.sync.dma_start(
                                out=vt[:, tt, :],
                                in_=v_d[128 * tt:128 * (tt + 1),
                                        128 * h:128 * (h + 1)])
                        for c in range(4):
                            nj = 4 * c + 4
                            ap = ps3.tile([128, 512], F32, tag="attn_ps")
                            sp = ps4.tile([1, 512], F32, tag="sum_ps")
                            for j in range(nj):
                                scp = ps2.tile([128, 512], F32, tag="sc_ps")
                                nc.tensor.matmul(
                                    scp[:], kt_[:, 128 * j:128 * (j + 1)],
                                    qt[:, 512 * c:512 * (c + 1)],
                                    start=True, stop=True)
                                pb = pr_pool.tile([128, 512], F32R, tag="probs")
                                nc.scalar.activation(
                                    out=pb[:], in_=scp[:],
                                    func=mybir.ActivationFunctionType.Exp,
                                    scale=1.0)
                                if j >= 4 * c:
                                    r = j - 4 * c
                                    nc.vector.tensor_mul(
                                        pb[:], pb[:],
                                        mask_t[:, 512 * r:512 * (r + 1)])
                                nc.tensor.matmul(sp[:], ones_t[:], pb[:],
                                                 start=(j == 0),
                                                 stop=(j == nj - 1))
                                nc.tensor.matmul(ap[:], vt[:, j, :], pb[:],
                                                 start=(j == 0),
                                                 stop=(j == nj - 1))
                            rc = sm_pool.tile([1, 512], F32, tag="rc")
                            nc.vector.reciprocal(rc[:], sp[:])
                            nc.sync.dma_start(
                                out=rcp_d[h:h + 1, 512 * c:512 * (c + 1)],
                                in_=rc[:])
                            rb = sm_pool.tile([128, 512], F32, tag="rb")
                            nc.sync.dma_start(
                                out=rb[:],
                                in_=rcp_d[h:h + 1, 512 * c:512 * (c + 1)]
                                .to_broadcast((128, 512)))
                            nc.vector.tensor_mul(
                                attn[h][:, 512 * c:512 * (c + 1)], ap[:],
                                rb[:])

                # ---------------- dense phase ----------------
                with tc.tile_pool(name="wd", bufs=2) as wd_pool, \
                     tc.tile_pool(name="oe", bufs=3) as oe_pool, \
                     tc.tile_pool(name="ps5", bufs=3, space="PSUM") as ps5:
                    ranges = []
                    for (t0, t1) in _token_tiles(0, S):
                        if t0 < NV < t1:
                            ranges.append((t0, NV, "V"))
                            ranges.append((NV, t1, "L"))
                        else:
                            ranges.append((t0, t1, "V" if t0 < NV else "L"))
                    for nh in range(2):
                        wdv_t = wd_pool.tile([128, HPC, 2048], F32R, tag="wdv")
                        wdl_t = wd_pool.tile([128, HPC, 2048], F32R, tag="wdl")
                        for hh in range(HPC):
                            nc.sync.dma_start(
                                out=wdv_t[:, hh, :],
                                in_=wd_v[128 * hh:128 * (hh + 1),
                                         2048 * nh:2048 * (nh + 1)])
                            nc.sync.dma_start(
                                out=wdl_t[:, hh, :],
                                in_=wd_l[128 * hh:128 * (hh + 1),
                                         2048 * nh:2048 * (nh + 1)])
                        for (t0, t1, e) in ranges:
                            mw = t1 - t0
                            wd_t = wdv_t if e == "V" else wdl_t
                            for n in range(4):
                                po = ps5.tile([128, 512], F32, tag="o_ps")
                                for hh in range(HPC):
                                    nc.tensor.matmul(
                                        po[:mw, :],
                                        attn[hh][:, t0:t1],
                                        wd_t[:, hh, 512 * n:512 * (n + 1)],
                                        start=(hh == 0), stop=(hh == HPC - 1))
                                ob = oe_pool.tile([128, 512], F32, tag="ob")
                                nc.vector.tensor_copy(ob[:mw, :], po[:mw, :])
                                nc.sync.dma_start(
                                    out=out_d[t0:t1,
                                              2048 * nh + 512 * n:
                                              2048 * nh + 512 * (n + 1)],
                                    in_=ob[:mw, :])
    nc.compile()
    return nc


def _prep_inputs(inputs):
    hs = np.ascontiguousarray(inputs["hidden_states"], np.float32)
    cos = np.asarray(inputs["cos"], np.float32)
    sin = np.asarray(inputs["sin"], np.float32)
    vi = np.asarray(inputs["vision_indices"]).ravel()
    li = np.asarray(inputs["language_indices"]).ravel()
    nv = vi.size
    assert nv == NV and np.array_equal(vi, np.arange(nv)) and \
        np.array_equal(li, np.arange(nv, S)), "unsupported index layout"
    hsT = np.ascontiguousarray(hs.T)
    scale = 1.0 / np.sqrt(np.float32(D))
    cosT = np.ascontiguousarray(cos.T)
    sinT = np.ascontiguousarray(sin.T)
    rmT = np.zeros((D, D), np.float32)
    for d in range(64):
        rmT[d + 64, d] = -1.0
        rmT[d, d + 64] = 1.0
    masks = np.zeros((128, 4 * 512), np.float32)
    tri = np.tril(np.ones((128, 128), np.float32)).T  # [t, s]: 1 iff t <= s
    for r in range(4):
        blk = np.ones((128, 512), np.float32)
        blk[:, :128 * r] = 0.0
        blk[:, 128 * r:128 * (r + 1)] = tri
        masks[:, 512 * r:512 * (r + 1)] = blk
    b = np.asarray(inputs["vision_qkv_b"], np.float32)
    in_maps = []
    for r in range(NCORES):
        h0 = HPC * r
        qc = slice(128 * h0, 128 * h0 + VC)
        kc = slice(HID + 128 * h0, HID + 128 * h0 + VC)
        vcs = slice(2 * HID + 128 * h0, 2 * HID + 128 * h0 + VC)
        wqk_vv = np.concatenate([inputs["vision_qkv_w"][:, qc],
                                 inputs["vision_qkv_w"][:, kc]], 1)
        wqk_ll = np.concatenate([inputs["lang_qkv_w"][:, qc],
                                 inputs["lang_qkv_w"][:, kc]], 1)
        bqk_r = np.concatenate([b[qc], b[kc]]).reshape(8, 128).T
        in_maps.append({
            "hsT": hsT,
            "wqk_v": np.ascontiguousarray(wqk_vv, np.float32),
            "wqk_l": np.ascontiguousarray(wqk_ll, np.float32),
            "wv_v": np.ascontiguousarray(inputs["vision_qkv_w"][:, vcs],
                                         np.float32),
            "wv_l": np.ascontiguousarray(inputs["lang_qkv_w"][:, vcs],
                                         np.float32),
            "wd_v": np.ascontiguousarray(
                inputs["vision_dense_w"][128 * h0:128 * h0 + VC, :],
                np.float32),
            "wd_l": np.ascontiguousarray(
                inputs["lang_dense_w"][128 * h0:128 * h0 + VC, :], np.float32),
            "bqk": np.ascontiguousarray(bqk_r, np.float32),
            "bv": np.ascontiguousarray(b[vcs].reshape(1, VC), np.float32),
            "cosq": cosT * scale, "sinq": sinT * scale,
            "cosk": cosT, "sink": sinT,
            "rmT": rmT, "ones": np.ones((128, 1), np.float32),
            "masks": masks,
        })
    return in_maps


def kernel(**inputs):
    if "nc" not in _CACHE:
        _CACHE["nc"] = _build()
    nc = _CACHE["nc"]
    in_maps = _prep_inputs(inputs)
    res = run_bass_kernel_spmd(nc, in_maps, list(range(NCORES)),
                               **_CACHE.get("run_kwargs", {}))
    _CACHE["last_results"] = res
    out = np.zeros((S, HID), np.float64)
    for r in range(NCORES):
        out += res.results[r]["out"].astype(np.float64)
    return out.astype(np.float32)



# revision 6
# speedup vs baseline: 1.7212x; 1.7212x over previous
"""Trainium2 Bass kernel for modality-routed (CogVLM-style) attention, v2.

Contract: kernel(**inputs) takes FULL unsharded numpy inputs (as produced by
the reference's setup_inputs) and returns the FULL [2048, 4096] fp32 output.

Sharding: tensor-parallel over heads. Core r owns heads 4r..4r+3:
  - qkv weights column-sharded; q/k/v all computed in transposed [dim, token]
    orientation via stationary weight tiles (12 uniform m-blocks per expert);
    v is then flipped to natural [token, dim] per 128-token tile with DMA
    transposes (bf16 XBAR path), no PE/PSUM cost.
  - dense weights row-sharded; each core emits a partial output in transposed
    [4096, 2048] orientation, summed + transposed on the host.

Everything between HBM input streams and the final output write stays in
SBUF (no DRAM round-trips for q/k/v/attn). All streams are bf16 (PE rate for
bf16 == fp32r here, but DMA bytes halve and SBUF residency becomes possible);
PSUM accumulation stays fp32. The 1/sqrt(D) score scale is folded into the
softmax exp's activation scale, RoPE rotate-half is one PE matmul with a
signed permutation matrix + two vector muls + one gpsimd add.

Emission interleaves token-segment QKV with attention chunks so the PE
stream never drains at phase boundaries:
  V-seg QKV -> attn c0 | L1 QKV -> attn c1 | L2 -> c2 | L3 -> c3 -> dense.
Expert routing is free: vision tokens are rows 0..575, so expert choice is
just which weight tile streams in for a given token chunk; the vision qkv
bias is a per-partition activation bias on the PSUM evacuation.
"""

import sys

import numpy as np

if "/opt/trn_rl_repo" not in sys.path:
    sys.path.insert(0, "/opt/trn_rl_repo")

import concourse.bass as bass  # noqa: E402,F401
import concourse.tile as tile  # noqa: E402
from concourse import bacc, mybir  # noqa: E402
from concourse.bass_utils import run_bass_kernel_spmd  # noqa: E402

S = 2048
HID = 4096
H = 32
D = 128
NCORES = 8
HPC = H // NCORES          # heads per core = 4
NV = 576                   # vision tokens occupy rows [0, NV)
NKT = HID // 128           # 32 K-tiles
NM = 3 * HPC               # 12 m-blocks (4 q, 4 k, 4 v)
QKSCALE = 1.0 / float(np.sqrt(D))

F32 = mybir.dt.float32
BF = mybir.dt.bfloat16

# token segments (start, end, expert): attention chunk c emits after seg c
SEGS = [(0, 576, 0), (576, 1088, 1), (1088, 1600, 1), (1600, 2048, 1)]
# dense token chunks with expert routing
DCH = [(0, 512, 0), (512, 576, 0), (576, 1088, 1), (1088, 1600, 1),
       (1600, 2048, 1)]

_CACHE = {}


def _chunks(s0, s1):
    out = []
    c = s0
    while c < s1:
        n = min(s1, c + 512)
        out.append((c, n))
        c = n
    return out


def _build():
    nc = bacc.Bacc("TRN2", target_bir_lowering=False, debug=False,
                   num_devices=NCORES)
    dti = nc.dram_tensor
    hs_d = dti("hs", [128, NKT * S], BF, kind="ExternalInput").ap()
    w_d = dti("w", [128, 2 * NM * NKT * 128], BF, kind="ExternalInput").ap()
    wd_d = dti("wd", [128, 2 * 32 * HPC * 128], BF, kind="ExternalInput").ap()
    cos_d = dti("cos", [128, S], BF, kind="ExternalInput").ap()
    sin_d = dti("sin", [128, S], BF, kind="ExternalInput").ap()
    mask_d = dti("mask", [128, 4 * 512], BF, kind="ExternalInput").ap()
    rm_d = dti("rm", [D, D], BF, kind="ExternalInput").ap()
    ones_d = dti("ones", [128, 1], BF, kind="ExternalInput").ap()
    bias_d = dti("bias", [128, NM], F32, kind="ExternalInput").ap()
    rcp_d = dti("rcp", [16, 512], F32).ap()             # broadcast bounce
    out_d = dti("outT", [HID, S], BF, kind="ExternalOutput").ap()

    with tile.TileContext(nc) as tc:
        with tc.tile_pool(name="glob", bufs=1) as glob:
            cos_t = glob.tile([128, S], BF)
            nc.sync.dma_start(out=cos_t[:], in_=cos_d[:])
            sin_t = glob.tile([128, S], BF)
            nc.sync.dma_start(out=sin_t[:], in_=sin_d[:])
            mask_t = glob.tile([128, 4, 512], BF)
            nc.sync.dma_start(out=mask_t[:], in_=mask_d[:])
            rm_t = glob.tile([D, D], BF)
            nc.sync.dma_start(out=rm_t[:], in_=rm_d[:])
            ones_t = glob.tile([128, 1], BF)
            nc.sync.dma_start(out=ones_t[:], in_=ones_d[:])
            bias_t = glob.tile([128, NM], F32)
            nc.sync.dma_start(out=bias_t[:], in_=bias_d[:])

            qT = [glob.tile([128, S], BF, name=f"qT{h}") for h in range(HPC)]
            kT = [glob.tile([128, S], BF, name=f"kT{h}") for h in range(HPC)]
            vT = [glob.tile([128, S], BF, name=f"vT{h}") for h in range(HPC)]
            v_sb = [glob.tile([128, 16, 128], BF, name=f"v{h}")
                    for h in range(HPC)]
            attnT = [glob.tile([128, S], BF, name=f"attnT{h}")
                     for h in range(HPC)]

            with tc.tile_pool(name="hsp", bufs=2) as hs_pool, \
                 tc.tile_pool(name="wp", bufs=2) as w_pool, \
                 tc.tile_pool(name="evp", bufs=3) as ev_pool, \
                 tc.tile_pool(name="pbp", bufs=4) as pb_pool, \
                 tc.tile_pool(name="smp", bufs=2) as sm_pool, \
                 tc.tile_pool(name="mmps", bufs=2, space="PSUM") as mm_ps, \
                 tc.tile_pool(name="scps", bufs=4, space="PSUM") as sc_ps, \
                 tc.tile_pool(name="spps", bufs=2, space="PSUM") as sp_ps:

                for si, (s0, s1, e) in enumerate(SEGS):
                    sw = s1 - s0
                    hst = hs_pool.tile([128, NKT, 576], BF, tag="hs")
                    for kt in range(NKT):
                        nc.sync.dma_start(
                            out=hst[:, kt, :sw],
                            in_=hs_d[:, kt * S + s0:kt * S + s1])
                    for m in range(NM):
                        wt = w_pool.tile([128, NKT, 128], BF, tag="w")
                        base = (e * NM + m) * NKT * 128
                        nc.sync.dma_start(out=wt[:],
                                          in_=w_d[:, base:base + NKT * 128])
                        for (c0, c1) in _chunks(s0, s1):
                            w2 = c1 - c0
                            o0 = c0 - s0
                            pt = mm_ps.tile([128, 512], F32, tag="mm")
                            for kt in range(NKT):
                                nc.tensor.matmul(pt[:, :w2], wt[:, kt, :],
                                                 hst[:, kt, o0:o0 + w2],
                                                 start=(kt == 0),
                                                 stop=(kt == NKT - 1))
                            if m < 2 * HPC:
                                qk_sb = ev_pool.tile([128, 512], BF,
                                                     tag="qksb")
                                if e == 0:
                                    nc.scalar.activation(
                                        out=qk_sb[:, :w2], in_=pt[:, :w2],
                                        func=mybir.ActivationFunctionType
                                        .Identity,
                                        bias=bias_t[:, m:m + 1], scale=1.0)
                                else:
                                    nc.scalar.activation(
                                        out=qk_sb[:, :w2], in_=pt[:, :w2],
                                        func=mybir.ActivationFunctionType
                                        .Copy, scale=1.0)
                                rot = mm_ps.tile([128, 512], F32, tag="mm")
                                nc.tensor.matmul(rot[:, :w2], rm_t[:],
                                                 qk_sb[:, :w2],
                                                 start=True, stop=True)
                                prod = ev_pool.tile([128, 512], BF,
                                                    tag="prod")
                                nc.vector.tensor_mul(prod[:, :w2],
                                                     qk_sb[:, :w2],
                                                     cos_t[:, c0:c1])
                                rp = ev_pool.tile([128, 512], BF, tag="rp")
                                nc.vector.tensor_mul(rp[:, :w2], rot[:, :w2],
                                                     sin_t[:, c0:c1])
                                tgt = qT[m] if m < HPC else kT[m - HPC]
                                nc.gpsimd.tensor_add(tgt[:, c0:c1],
                                                     prod[:, :w2],
                                                     rp[:, :w2])
                            else:
                                mv = m - 2 * HPC
                                if e == 0:
                                    nc.scalar.activation(
                                        out=vT[mv][:, c0:c1], in_=pt[:, :w2],
                                        func=mybir.ActivationFunctionType
                                        .Identity,
                                        bias=bias_t[:, m:m + 1], scale=1.0)
                                else:
                                    nc.scalar.activation(
                                        out=vT[mv][:, c0:c1], in_=pt[:, :w2],
                                        func=mybir.ActivationFunctionType
                                        .Copy, scale=1.0)

                    # ---- v tiles for this attention chunk: DMA transpose
                    c = si
                    for h in range(HPC):
                        for jt in range(4 * c, 4 * c + 4):
                            nc.sync.dma_start_transpose(
                                out=v_sb[h][:, jt, :],
                                in_=vT[h][:, 128 * jt:128 * (jt + 1)])

                    # ---- attention chunk c (queries 512c .. 512c+512)
                    q0 = 512 * c
                    nj = 4 * (c + 1)
                    for h in range(HPC):
                        ap = mm_ps.tile([128, 512], F32, tag="mm")
                        sp = sp_ps.tile([1, 512], F32, tag="sp")
                        pend = []

                        def flush(j, pb, ap=ap, sp=sp, h=h, nj=nj):
                            nc.tensor.matmul(sp[:], ones_t[:], pb[:],
                                             start=(j == 0),
                                             stop=(j == nj - 1))
                            nc.tensor.matmul(ap[:], v_sb[h][:, j, :], pb[:],
                                             start=(j == 0),
                                             stop=(j == nj - 1))

                        for j in range(nj):
                            scp = sc_ps.tile([128, 512], F32, tag="sc")
                            nc.tensor.matmul(
                                scp[:], kT[h][:, 128 * j:128 * (j + 1)],
                                qT[h][:, q0:q0 + 512],
                                start=True, stop=True)
                            pb = pb_pool.tile([128, 512], BF, tag="pb")
                            nc.scalar.activation(
                                out=pb[:], in_=scp[:],
                                func=mybir.ActivationFunctionType.Exp,
                                scale=QKSCALE)
                            if j >= 4 * c:
                                r = j - 4 * c
                                nc.gpsimd.tensor_mul(pb[:], pb[:],
                                                     mask_t[:, r, :])
                            pend.append((j, pb))
                            if len(pend) > 2:
                                flush(*pend.pop(0))
                        for it in pend:
                            flush(*it)

                        hc = 4 * c + h
                        rc = sm_pool.tile([1, 512], F32, tag="rc")
                        nc.vector.reciprocal(rc[:], sp[:])
                        nc.sync.dma_start(out=rcp_d[hc:hc + 1, :], in_=rc[:])
                        rb = sm_pool.tile([128, 512], F32, tag="rb")
                        nc.sync.dma_start(
                            out=rb[:],
                            in_=rcp_d[hc:hc + 1, :].to_broadcast((128, 512)))
                        nc.vector.tensor_mul(attnT[h][:, q0:q0 + 512],
                                             ap[:], rb[:])

            # ---------------- dense phase ----------------
            with tc.tile_pool(name="wdp", bufs=4) as wd_pool, \
                 tc.tile_pool(name="oep", bufs=4) as oe_pool, \
                 tc.tile_pool(name="dnps", bufs=4, space="PSUM") as dn_ps:
                evac_eng = [lambda o, i: nc.scalar.activation(
                                out=o, in_=i,
                                func=mybir.ActivationFunctionType.Copy,
                                scale=1.0),
                            nc.vector.tensor_copy]
                ei = 0
                for o in range(32):
                    wde = []
                    for e in range(2):
                        wdt = wd_pool.tile([128, HPC, 128], BF, tag="wd")
                        base = (e * 32 + o) * HPC * 128
                        nc.sync.dma_start(
                            out=wdt[:], in_=wd_d[:, base:base + HPC * 128])
                        wde.append(wdt)
                    for (t0, t1, e) in DCH:
                        w2 = t1 - t0
                        po = dn_ps.tile([128, 512], F32, tag="po")
                        for hh in range(HPC):
                            nc.tensor.matmul(po[:, :w2], wde[e][:, hh, :],
                                             attnT[hh][:, t0:t1],
                                             start=(hh == 0),
                                             stop=(hh == HPC - 1))
                        oe = oe_pool.tile([128, 512], BF, tag="oe")
                        evac_eng[ei % 2](oe[:, :w2], po[:, :w2])
                        ei += 1
                        nc.sync.dma_start(
                            out=out_d[128 * o:128 * (o + 1), t0:t1],
                            in_=oe[:, :w2])
    nc.compile()
    return nc


def _prep_inputs(inputs):
    import ml_dtypes
    bf = ml_dtypes.bfloat16

    hs = np.asarray(inputs["hidden_states"], np.float32)
    cos = np.asarray(inputs["cos"], np.float32)
    sin = np.asarray(inputs["sin"], np.float32)
    vi = np.asarray(inputs["vision_indices"]).ravel()
    li = np.asarray(inputs["language_indices"]).ravel()
    assert vi.size == NV and np.array_equal(vi, np.arange(NV)) and \
        np.array_equal(li, np.arange(NV, S)), "unsupported index layout"

    # hs tiled [128, (kt, t)]
    hs_t = np.ascontiguousarray(
        hs.T.reshape(NKT, 128, S).transpose(1, 0, 2).reshape(128, NKT * S)
    ).astype(bf)

    cos_t = np.ascontiguousarray(cos.T).astype(bf)
    sin_t = np.ascontiguousarray(sin.T).astype(bf)

    rm = np.zeros((D, D), np.float32)
    for d in range(64):
        rm[d + 64, d] = -1.0
        rm[d, d + 64] = 1.0
    rm = rm.astype(bf)

    # mask[p, r, q'] = 1 iff 128 r + p <= q'
    p = np.arange(128)[:, None, None]
    r = np.arange(4)[None, :, None]
    q = np.arange(512)[None, None, :]
    mask = (128 * r + p <= q).astype(np.float32).reshape(128, 4 * 512)
    mask = mask.astype(bf)

    ones = np.ones((128, 1), np.float32).astype(bf)

    b = np.asarray(inputs["vision_qkv_b"], np.float32)
    Wqkv = np.stack([np.asarray(inputs["vision_qkv_w"], np.float32),
                     np.asarray(inputs["lang_qkv_w"], np.float32)])
    Wd = np.stack([np.asarray(inputs["vision_dense_w"], np.float32),
                   np.asarray(inputs["lang_dense_w"], np.float32)])

    in_maps = []
    for rr in range(NCORES):
        q0 = 512 * rr
        cols = np.r_[q0:q0 + 512, HID + q0:HID + q0 + 512,
                     2 * HID + q0:2 * HID + q0 + 512]
        # w tiled [128, (e, m, kt, c)]
        wc = Wqkv[:, :, cols]                                # [2, 4096, 1536]
        w_t = np.ascontiguousarray(
            wc.reshape(2, NKT, 128, NM, 128)
              .transpose(2, 0, 3, 1, 4)
              .reshape(128, 2 * NM * NKT * 128)).astype(bf)
        # wd tiled [128, (e, o, hh, c)]
        wdc = Wd[:, q0:q0 + 512, :]                          # [2, 512, 4096]
        wd_t = np.ascontiguousarray(
            wdc.reshape(2, HPC, 128, 32, 128)
               .transpose(2, 0, 3, 1, 4)
               .reshape(128, 2 * 32 * HPC * 128)).astype(bf)
        bias_t = np.ascontiguousarray(
            b[cols].reshape(NM, 128).T).astype(np.float32)
        in_maps.append({
            "hs": hs_t, "w": w_t, "wd": wd_t,
            "cos": cos_t, "sin": sin_t, "mask": mask, "rm": rm,
            "ones": ones, "bias": bias_t,
        })
    return in_maps


def kernel(**inputs):
    if "nc" not in _CACHE:
        _CACHE["nc"] = _build()
    nc = _CACHE["nc"]
    in_maps = _prep_inputs(inputs)
    res = run_bass_kernel_spmd(nc, in_maps, list(range(NCORES)),
                               **_CACHE.get("run_kwargs", {}))
    _CACHE["last_results"] = res
    out = np.zeros((HID, S), np.float32)
    for r in range(NCORES):
        out += res.results[r]["outT"].astype(np.float32)
    return np.ascontiguousarray(out.T)
